# revision 13
# baseline (speedup 1.0000x reference)
"""Trainium2 Bass kernel for ModalityAttentionLayer.

Reference computation (the noise branch is multiplied by 0.0, so it is
dead code):
    pooled[b,c] = mean over (H,W) of x[b,c]
    h = relu(pooled @ w1.T + b1)           # [B, 16]
    gate = sigmoid(h @ w2.T + b2)          # [B, C]
    w = 0.1 + 0.9 * gate
    out = (x * w[:, :, None, None], w[:, :, None, None])

Sharding: pure data parallel over batch, 2 samples per core on 8 cores.
Per core the 8 MiB shard is held SBUF-resident so HBM traffic is one
read + one write of the shard (~16.8 MB, the per-core HBM-share
roofline).

Device schedule per core:
    x[s] viewed as [128, 8192] (partition p = 32KiB contiguous; channel c
    owns partitions 32c..32c+31), loaded in 4 chunks of [128, 2048] per
    sample. All 16 big DMAs ride the Sync HWDGE ring (one ring sustains
    ~420-450 GB/s; two concurrent rings arbitrate down to ~330). Chunk
    partial sums: sample 0 splits each chunk between vector reduce and a
    scalar-engine Copy+accum; sample 1 reduces whole chunks on scalar so
    the vector engine is free for the store multiplies. The mean-matrix
    and first MLP layer fold into one matmul (G1[p,o] = w1[o,c(p)]/HW)
    contracting partial sums over partitions; b1/b2 ride the relu /
    sigmoid per-partition bias; a final matmul against [0.9*mask;
    0.1*ones] broadcasts w = 0.9*gate + 0.1 to all 128 partitions.
    Chunks are scaled on vector and stored via the sync ring. A dummy
    sigmoid pre-warms the scalar activation table.
"""

import numpy as np

import concourse.bacc as bacc
import concourse.mybir as mybir
from concourse import tile
from concourse.bass_utils import run_bass_kernel_spmd

N_CORES = 8
B, C, H, W = 16, 4, 512, 512
BS = B // N_CORES          # samples per core
HW = H * W
P = 128
FREE = C * HW // P         # 8192 free elems per sample
NCHUNK = 4
CHUNK = FREE // NCHUNK     # 2048
HALF = CHUNK // 2
PPC = P // C               # partitions per channel image
HID = 16                   # MLP hidden dim

_NC_CACHE = {}


def _build_nc():
    f32 = mybir.dt.float32
    AF = mybir.ActivationFunctionType

    nc = bacc.Bacc(
        "TRN2", target_bir_lowering=False, debug=False, num_devices=N_CORES,
        enable_partition_id=False,
    )

    x_d = nc.dram_tensor("x", [BS, P, FREE], f32, kind="ExternalInput")
    g1_d = nc.dram_tensor("g1", [P, HID], f32, kind="ExternalInput")
    b1c_d = nc.dram_tensor("b1c", [HID, 1], f32, kind="ExternalInput")
    w2t_d = nc.dram_tensor("w2t", [HID, C], f32, kind="ExternalInput")
    b2c_d = nc.dram_tensor("b2c", [C, 1], f32, kind="ExternalInput")
    # [5, 128]: rows 0..3 = 0.9 * channel mask, row 4 = 0.1 (floor)
    gmTs_d = nc.dram_tensor("gmTs", [C + 1, P], f32, kind="ExternalInput")
    y_d = nc.dram_tensor("y", [BS, P, FREE], f32, kind="ExternalOutput")
    wout_d = nc.dram_tensor("wout", [BS, C, 1], f32, kind="ExternalOutput")

    with tile.TileContext(nc) as tc:
        with (
            tc.tile_pool(name="consts", bufs=1) as cpool,
            tc.tile_pool(name="xres", bufs=1) as xpool,
            tc.tile_pool(name="small", bufs=1) as spool,
            tc.tile_pool(name="ochunk", bufs=8) as opool,
            tc.tile_pool(name="psum", bufs=1, space="PSUM") as ppool,
        ):
            # big loads issue first on the sync ring, nothing ahead of them
            xts = []
            for s in range(BS):
                xt = xpool.tile([P, FREE], f32, tag=f"x{s}", name=f"x{s}")
                xts.append(xt)
                for j in range(NCHUNK):
                    sl = slice(j * CHUNK, (j + 1) * CHUNK)
                    nc.sync.dma_start(xt[:, sl], x_d[s, :, sl])

            # pre-warm the scalar activation table (sigmoid) so the
            # one-time ACT_TABLE_LOAD is off the MLP critical path
            warm = spool.tile([1, 1], f32, tag="warm", name="warm")
            nc.scalar.activation(warm[:], nc.const_aps.tensor(0.0, (1, 1)), AF.Sigmoid)

            # tiny constants on the scalar ring (sync ring is loads-only)
            g1 = cpool.tile([P, HID], f32)
            nc.scalar.dma_start(g1[:], g1_d[:])
            b1c = cpool.tile([HID, 1], f32)
            nc.scalar.dma_start(b1c[:], b1c_d[:])
            w2t = cpool.tile([HID, C], f32)
            nc.scalar.dma_start(w2t[:], w2t_d[:])
            b2c = cpool.tile([C, 1], f32)
            nc.scalar.dma_start(b2c[:], b2c_d[:])
            gmTs = cpool.tile([C + 1, P], f32)
            nc.scalar.dma_start(gmTs[:], gmTs_d[:])

            # gate5 row 4 must be 1.0 (multiplies the 0.1 floor row);
            # memset early on gpsimd, which is otherwise idle
            gate5s = []
            for s in range(BS):
                gate5 = spool.tile([C + 1, 1], f32, tag=f"gate5{s}", name=f"gate5{s}")
                nc.gpsimd.memset(gate5[:, :], 1.0)
                gate5s.append(gate5)

            partials_t = []
            for s in range(BS):
                partials = spool.tile(
                    [P, 2 * NCHUNK], f32, tag=f"partials{s}", name=f"partials{s}"
                )
                partials_t.append(partials)

            # sample 0 chunk partial sums: vector half + scalar half each
            for j in range(NCHUNK):
                lo = j * CHUNK
                nc.vector.reduce_sum(
                    partials_t[0][:, 2 * j : 2 * j + 1],
                    xts[0][:, lo : lo + HALF],
                    axis=mybir.AxisListType.X,
                )
                scratch = spool.tile(
                    [P, HALF], f32, tag="scratch", name=f"scr0_{j}", bufs=2
                )
                nc.scalar.activation(
                    scratch[:],
                    xts[0][:, lo + HALF : lo + CHUNK],
                    AF.Copy,
                    accum_out=partials_t[0][:, 2 * j + 1 : 2 * j + 2],
                )
                # sample 1 chunks j=0..2 reduce fully on scalar, keeping
                # vector free for sample-0 store multiplies later
                if j < NCHUNK - 1:
                    scr1 = spool.tile(
                        [P, CHUNK], f32, tag="scratch1", name=f"scr1_{j}", bufs=2
                    )
                    nc.scalar.activation(
                        scr1[:],
                        xts[1][:, lo : lo + CHUNK],
                        AF.Copy,
                        accum_out=partials_t[1][:, 2 * j : 2 * j + 1],
                    )

            def mlp_and_stores(s, mul_engines):
                partials = partials_t[s]
                gate5 = gate5s[s]
                xt = xts[s]
                p1 = spool.tile([P, 1], f32, tag=f"p1{s}", name=f"p1{s}")
                nc.vector.reduce_sum(p1[:], partials[:], axis=mybir.AxisListType.X)

                # z1[o] = sum_p G1[p,o] * p1[p]; h = relu(z1 + b1)
                z1_ps = ppool.tile([HID, 1], f32, tag="z1", name=f"z1{s}")
                nc.tensor.matmul(z1_ps[:], g1[:], p1[:], start=True, stop=True)
                hT = spool.tile([HID, 1], f32, tag=f"hT{s}", name=f"hT{s}")
                nc.scalar.activation(hT[:], z1_ps[:], AF.Relu, bias=b1c[:])

                # z2[c] = sum_o w2[c,o]*h[o]; gate = sigmoid(z2 + b2)
                z2_ps = ppool.tile([C, 1], f32, tag="z2", name=f"z2{s}")
                nc.tensor.matmul(z2_ps[:], w2t[:], hT[:], start=True, stop=True)
                nc.scalar.activation(gate5[0:C, :], z2_ps[:], AF.Sigmoid, bias=b2c[:])

                # broadcast: wps[p] = 0.9*gate[c(p)] + 0.1 in one matmul
                wps = ppool.tile([P, 1], f32, tag="wps", name=f"wps{s}")
                nc.tensor.matmul(wps[:], gmTs[:], gate5[:], start=True, stop=True)
                wb = spool.tile([P, 1], f32, tag=f"wb{s}", name=f"wb{s}")
                nc.scalar.copy(wb[:], wps[:])

                # weights output [C,1]: one value per 32-partition group
                nc.scalar.dma_start(wout_d[s], wb[0:P:PPC, :])

                for j in range(NCHUNK):
                    sl = slice(j * CHUNK, (j + 1) * CHUNK)
                    ot = opool.tile([P, CHUNK], f32, tag="ochunk", name=f"o{s}_{j}")
                    eng = mul_engines[j % len(mul_engines)]
                    if eng == "v":
                        nc.vector.tensor_scalar_mul(ot[:], xt[:, sl], wb[:])
                    else:
                        nc.scalar.mul(ot[:], xt[:, sl], wb[:])
                    nc.sync.dma_start(y_d[s, :, sl], ot[:])

            # sample 0: MLP + stores (muls on vector; scalar still owns
            # sample-1 chunk reduces)
            mlp_and_stores(0, mul_engines=["v", "v"])

            # sample 1 last chunk: split vector/scalar so it finishes fast
            lo = 3 * CHUNK
            nc.vector.reduce_sum(
                partials_t[1][:, 6:7],
                xts[1][:, lo : lo + HALF],
                axis=mybir.AxisListType.X,
            )
            scr1 = spool.tile([P, HALF], f32, tag="scratch", name="scr1_3", bufs=2)
            nc.scalar.activation(
                scr1[:],
                xts[1][:, lo + HALF : lo + CHUNK],
                AF.Copy,
                accum_out=partials_t[1][:, 7:8],
            )
            mlp_and_stores(1, mul_engines=["v", "s"])

    nc.compile()
    return nc


def _get_nc():
    if "nc" not in _NC_CACHE:
        _NC_CACHE["nc"] = _build_nc()
    return _NC_CACHE["nc"]


def kernel(x, w1, b1, w2, b2, embed_w=None, affine_a=None, affine_b=None, **_unused):
    x = np.ascontiguousarray(np.asarray(x, dtype=np.float32))
    w1m = np.asarray(w1, np.float32).reshape(4 * C, C)      # [16, 4]
    b1v = np.asarray(b1, np.float32).reshape(4 * C)
    w2m = np.asarray(w2, np.float32).reshape(C, 4 * C)      # [4, 16]
    b2v = np.asarray(b2, np.float32).reshape(C)

    pidx = np.arange(P) // PPC
    g1 = np.ascontiguousarray(w1m.T[pidx] / HW)             # [128, 16]
    b1c = np.ascontiguousarray(b1v[:, None])                # [16, 1]
    w2t = np.ascontiguousarray(w2m.T)                       # [16, 4]
    b2c = np.ascontiguousarray(b2v[:, None])                # [4, 1]
    gmTs = np.zeros((C + 1, P), np.float32)
    gmTs[pidx, np.arange(P)] = 0.9
    gmTs[C, :] = 0.1

    nc = _get_nc()
    xs = x.reshape(N_CORES, BS, P, FREE)
    in_maps = [
        {
            "x": np.ascontiguousarray(xs[i]),
            "g1": g1,
            "b1c": b1c,
            "w2t": w2t,
            "b2c": b2c,
            "gmTs": gmTs,
        }
        for i in range(N_CORES)
    ]
    res = run_bass_kernel_spmd(nc, in_maps, core_ids=list(range(N_CORES)))
    y = np.stack([res.results[i]["y"] for i in range(N_CORES)], axis=0)
    wo = np.stack([res.results[i]["wout"] for i in range(N_CORES)], axis=0)
    out = y.reshape(B, C, H, W)
    weights4 = wo.reshape(B, C, 1, 1)
    return out, weights4


# revision 16
# speedup vs baseline: 1.0271x; 1.0271x over previous
"""Trainium2 Bass kernel for ModalityAttentionLayer.

Reference computation (the noise branch is multiplied by 0.0, so it is
dead code):
    pooled[b,c] = mean over (H,W) of x[b,c]
    h = relu(pooled @ w1.T + b1)           # [B, 16]
    gate = sigmoid(h @ w2.T + b2)          # [B, C]
    w = 0.1 + 0.9 * gate
    out = (x * w[:, :, None, None], w[:, :, None, None])

Sharding: pure data parallel over batch, 2 samples per core on 8 cores.
Per core the 8 MiB shard is held SBUF-resident so HBM traffic is one
read + one write of the shard (~16.8 MB, the per-core HBM-share
roofline).

Device schedule per core:
    x[s] viewed as [128, 8192] (partition p = 32KiB contiguous; channel c
    owns partitions 32c..32c+31), loaded in 4 chunks of [128, 2048] per
    sample. All 16 big DMAs ride the Sync HWDGE ring (one ring sustains
    ~420-450 GB/s; two concurrent rings arbitrate down to ~330). Chunk
    partial sums: sample 0 splits each chunk between vector reduce and a
    scalar-engine Copy+accum; sample 1 reduces whole chunks on scalar so
    the vector engine is free for the store multiplies. The mean-matrix
    and first MLP layer fold into one matmul (G1[p,o] = w1[o,c(p)]/HW)
    contracting partial sums over partitions; b1/b2 ride the relu /
    sigmoid per-partition bias; a final matmul against [0.9*mask;
    0.1*ones] broadcasts w = 0.9*gate + 0.1 to all 128 partitions.
    Chunks are scaled on vector and stored via the sync ring. A dummy
    sigmoid pre-warms the scalar activation table.
"""

import numpy as np

import concourse.bacc as bacc
import concourse.mybir as mybir
from concourse import tile
from concourse.bass_utils import run_bass_kernel_spmd

N_CORES = 8
B, C, H, W = 16, 4, 512, 512
BS = B // N_CORES          # samples per core
HW = H * W
P = 128
FREE = C * HW // P         # 8192 free elems per sample
NCHUNK = 2
CHUNK = FREE // NCHUNK     # 4096
HALF = CHUNK // 2
PPC = P // C               # partitions per channel image
HID = 16                   # MLP hidden dim

_NC_CACHE = {}


def _build_nc():
    f32 = mybir.dt.float32
    AF = mybir.ActivationFunctionType

    nc = bacc.Bacc(
        "TRN2", target_bir_lowering=False, debug=False, num_devices=N_CORES,
        enable_partition_id=False,
    )

    x_d = nc.dram_tensor("x", [BS, P, FREE], f32, kind="ExternalInput")
    g1_d = nc.dram_tensor("g1", [P, HID], f32, kind="ExternalInput")
    b1c_d = nc.dram_tensor("b1c", [HID, 1], f32, kind="ExternalInput")
    w2t_d = nc.dram_tensor("w2t", [HID, C], f32, kind="ExternalInput")
    b2c_d = nc.dram_tensor("b2c", [C, 1], f32, kind="ExternalInput")
    # [5, 128]: rows 0..3 = 0.9 * channel mask, row 4 = 0.1 (floor)
    gmTs_d = nc.dram_tensor("gmTs", [C + 1, P], f32, kind="ExternalInput")
    y_d = nc.dram_tensor("y", [BS, P, FREE], f32, kind="ExternalOutput")
    wout_d = nc.dram_tensor("wout", [BS, C, 1], f32, kind="ExternalOutput")

    with tile.TileContext(nc) as tc:
        with (
            tc.tile_pool(name="consts", bufs=1) as cpool,
            tc.tile_pool(name="xres", bufs=1) as xpool,
            tc.tile_pool(name="small", bufs=1) as spool,
            tc.tile_pool(name="ochunk", bufs=2 * NCHUNK) as opool,
            tc.tile_pool(name="psum", bufs=1, space="PSUM") as ppool,
        ):
            # big loads issue first on the sync ring, nothing ahead of them
            xts = []
            for s in range(BS):
                xt = xpool.tile([P, FREE], f32, tag=f"x{s}", name=f"x{s}")
                xts.append(xt)
                for j in range(NCHUNK):
                    sl = slice(j * CHUNK, (j + 1) * CHUNK)
                    nc.sync.dma_start(xt[:, sl], x_d[s, :, sl])

            # pre-warm the scalar activation table (sigmoid) so the
            # one-time ACT_TABLE_LOAD is off the MLP critical path
            warm = spool.tile([1, 1], f32, tag="warm", name="warm")
            nc.scalar.activation(warm[:], nc.const_aps.tensor(0.0, (1, 1)), AF.Sigmoid)

            # tiny constants on the scalar ring (sync ring is loads-only)
            g1 = cpool.tile([P, HID], f32)
            nc.scalar.dma_start(g1[:], g1_d[:])
            b1c = cpool.tile([HID, 1], f32)
            nc.scalar.dma_start(b1c[:], b1c_d[:])
            w2t = cpool.tile([HID, C], f32)
            nc.scalar.dma_start(w2t[:], w2t_d[:])
            b2c = cpool.tile([C, 1], f32)
            nc.scalar.dma_start(b2c[:], b2c_d[:])
            gmTs = cpool.tile([C + 1, P], f32)
            nc.scalar.dma_start(gmTs[:], gmTs_d[:])

            # gate5 row 4 must be 1.0 (multiplies the 0.1 floor row);
            # memset early on gpsimd, which is otherwise idle
            gate5s = []
            for s in range(BS):
                gate5 = spool.tile([C + 1, 1], f32, tag=f"gate5{s}", name=f"gate5{s}")
                nc.gpsimd.memset(gate5[:, :], 1.0)
                gate5s.append(gate5)

            partials_t = []
            for s in range(BS):
                partials = spool.tile(
                    [P, 2 * NCHUNK], f32, tag=f"partials{s}", name=f"partials{s}"
                )
                partials_t.append(partials)

            # sample 0 chunk partial sums: vector half + scalar half each
            for j in range(NCHUNK):
                lo = j * CHUNK
                nc.vector.reduce_sum(
                    partials_t[0][:, 2 * j : 2 * j + 1],
                    xts[0][:, lo : lo + HALF],
                    axis=mybir.AxisListType.X,
                )
                scratch = spool.tile(
                    [P, HALF], f32, tag="scratch", name=f"scr0_{j}", bufs=2
                )
                nc.scalar.activation(
                    scratch[:],
                    xts[0][:, lo + HALF : lo + CHUNK],
                    AF.Copy,
                    accum_out=partials_t[0][:, 2 * j + 1 : 2 * j + 2],
                )
                # sample 1 chunks j=0..2 reduce fully on scalar, keeping
                # vector free for sample-0 store multiplies later
                if j < NCHUNK - 1:
                    scr1 = spool.tile(
                        [P, CHUNK], f32, tag="scratch1", name=f"scr1_{j}", bufs=2
                    )
                    nc.scalar.activation(
                        scr1[:],
                        xts[1][:, lo : lo + CHUNK],
                        AF.Copy,
                        accum_out=partials_t[1][:, 2 * j : 2 * j + 1],
                    )

            def mlp_and_stores(s, mul_engines):
                partials = partials_t[s]
                gate5 = gate5s[s]
                xt = xts[s]
                p1 = spool.tile([P, 1], f32, tag=f"p1{s}", name=f"p1{s}")
                nc.vector.reduce_sum(p1[:], partials[:], axis=mybir.AxisListType.X)

                # z1[o] = sum_p G1[p,o] * p1[p]; h = relu(z1 + b1)
                z1_ps = ppool.tile([HID, 1], f32, tag="z1", name=f"z1{s}")
                nc.tensor.matmul(z1_ps[:], g1[:], p1[:], start=True, stop=True)
                hT = spool.tile([HID, 1], f32, tag=f"hT{s}", name=f"hT{s}")
                nc.scalar.activation(hT[:], z1_ps[:], AF.Relu, bias=b1c[:])

                # z2[c] = sum_o w2[c,o]*h[o]; gate = sigmoid(z2 + b2)
                z2_ps = ppool.tile([C, 1], f32, tag="z2", name=f"z2{s}")
                nc.tensor.matmul(z2_ps[:], w2t[:], hT[:], start=True, stop=True)
                nc.scalar.activation(gate5[0:C, :], z2_ps[:], AF.Sigmoid, bias=b2c[:])

                # broadcast: wps[p] = 0.9*gate[c(p)] + 0.1 in one matmul
                wps = ppool.tile([P, 1], f32, tag="wps", name=f"wps{s}")
                nc.tensor.matmul(wps[:], gmTs[:], gate5[:], start=True, stop=True)
                wb = spool.tile([P, 1], f32, tag=f"wb{s}", name=f"wb{s}")
                nc.scalar.copy(wb[:], wps[:])

                # weights output [C,1]: one value per 32-partition group
                nc.scalar.dma_start(wout_d[s], wb[0:P:PPC, :])

                for j in range(NCHUNK):
                    sl = slice(j * CHUNK, (j + 1) * CHUNK)
                    ot = opool.tile([P, CHUNK], f32, tag="ochunk", name=f"o{s}_{j}")
                    eng = mul_engines[j % len(mul_engines)]
                    if eng == "v":
                        nc.vector.tensor_scalar_mul(ot[:], xt[:, sl], wb[:])
                    else:
                        nc.scalar.mul(ot[:], xt[:, sl], wb[:])
                    nc.sync.dma_start(y_d[s, :, sl], ot[:])

            # sample 0: MLP + stores (muls on vector; scalar still owns
            # sample-1 chunk reduces)
            mlp_and_stores(0, mul_engines=["v", "v"])

            # sample 1 last chunk: split vector/scalar so it finishes fast
            lo = (NCHUNK - 1) * CHUNK
            nc.vector.reduce_sum(
                partials_t[1][:, 2 * NCHUNK - 2 : 2 * NCHUNK - 1],
                xts[1][:, lo : lo + HALF],
                axis=mybir.AxisListType.X,
            )
            scr1 = spool.tile([P, HALF], f32, tag="scratch", name="scr1_3", bufs=2)
            nc.scalar.activation(
                scr1[:],
                xts[1][:, lo + HALF : lo + CHUNK],
                AF.Copy,
                accum_out=partials_t[1][:, 2 * NCHUNK - 1 : 2 * NCHUNK],
            )
            mlp_and_stores(1, mul_engines=["v", "s"])

    nc.compile()
    return nc


def _get_nc():
    if "nc" not in _NC_CACHE:
        _NC_CACHE["nc"] = _build_nc()
    return _NC_CACHE["nc"]


def kernel(x, w1, b1, w2, b2, embed_w=None, affine_a=None, affine_b=None, **_unused):
    x = np.ascontiguousarray(np.asarray(x, dtype=np.float32))
    w1m = np.asarray(w1, np.float32).reshape(4 * C, C)      # [16, 4]
    b1v = np.asarray(b1, np.float32).reshape(4 * C)
    w2m = np.asarray(w2, np.float32).reshape(C, 4 * C)      # [4, 16]
    b2v = np.asarray(b2, np.float32).reshape(C)

    pidx = np.arange(P) // PPC
    g1 = np.ascontiguousarray(w1m.T[pidx] / HW)             # [128, 16]
    b1c = np.ascontiguousarray(b1v[:, None])                # [16, 1]
    w2t = np.ascontiguousarray(w2m.T)                       # [16, 4]
    b2c = np.ascontiguousarray(b2v[:, None])                # [4, 1]
    gmTs = np.zeros((C + 1, P), np.float32)
    gmTs[pidx, np.arange(P)] = 0.9
    gmTs[C, :] = 0.1

    nc = _get_nc()
    xs = x.reshape(N_CORES, BS, P, FREE)
    in_maps = [
        {
            "x": np.ascontiguousarray(xs[i]),
            "g1": g1,
            "b1c": b1c,
            "w2t": w2t,
            "b2c": b2c,
            "gmTs": gmTs,
        }
        for i in range(N_CORES)
    ]
    res = run_bass_kernel_spmd(nc, in_maps, core_ids=list(range(N_CORES)))
    y = np.stack([res.results[i]["y"] for i in range(N_CORES)], axis=0)
    wo = np.stack([res.results[i]["wout"] for i in range(N_CORES)], axis=0)
    out = y.reshape(B, C, H, W)
    weights4 = wo.reshape(B, C, 1, 1)
    return out, weights4
